# revision 1
# baseline (speedup 1.0000x reference)
"""TRN2 Bass kernel for nn_Attention_369367187796.

Reference computation (B=4, DX=1024, N=4096, DQ=DK=DV=1024, fp32):
    Q = Wq @ x[b]; K = Wk @ x[b]; V = Wv @ x[b]          (per batch)
    scores = Q @ K.T   (contract n)
    p = softmax(scores / sqrt(DQ), axis=q)               <- softmax over q!
    out[q,n] = sum_k p[q,k] V[k,n]

Sharding: 8 cores = 4 batches x 2 dk-halves. Each core computes, for its
(batch b, k-half h): the full Q, its half of K and V, scoresT[k_half, q]
(softmax over q is the free axis -> fully local), and the partial
out[q, n] = sum_{k in half} p[k,q] V[k,n]. Host sums the two partials.

Precision strategy (HW-validated, end-to-end rel err 6.2e-4 vs fp64):
  - float32r (fp32 rounded to 11 mantissa bits) matmuls run at full PE rate;
    native fp32 runs at 1/4 rate; bf16 alone flips softmax argmaxes (logits
    have std ~740 with top-2 gaps down to ~0.4).
  - Q/K projections: single-pass f32r on mean-removed weights. The host
    ships Wq/Wk minus 0.5 (zero-mean entries -> incoherent rounding); the
    exact mean term 0.5*colsum(x)[n] is restored by one K=1 matmul per
    projection psum, whose own rounding error is constant across q and
    cancels exactly in the softmax.
  - scores: Q evicted as f32r hi+lo, K single f32r -> 2-pass split matmul
    (K-side rounding is largely common-mode per softmax row).
  - V projection and p@V: single-pass f32r (errors pass through softmax
    un-amplified); V stays SBUF-resident, no spill.

Layouts (per core):
  QT (n, q) and KT (n, k) are computed transposed so the scores matmul
  contracts n on partitions and softmax lands on the free axis:
    QT[n,q] = sum_d x[d,n] WqT[d,q]   lhsT = x-tile [d,n], rhs = WqT [d,q]
    scoresT[k,q]: lhsT = KT [n,k], rhs = QT [n,q]
    out[q,n]:     lhsT = pT [k,q],  rhs = V  [k,n]
  QT/KT spill to DRAM between phases as 4-tile-grouped tensors (fine-grained
  deps let phase 2 start while phase 1 is still projecting later n-tiles).
  The walrus verifier requires f32r matmul operands to be produced by a
  rounding compute op (not DMA), so reloaded spills are re-rounded with a
  cheap f32r->f32r tensor_copy.
"""

import math

import numpy as np

B_FULL, DX_FULL, N_FULL = 4, 1024, 4096
DQ_FULL = DK_FULL = 1024
N_CORES = 8

# precision mode for the Q/K projections: "f32r" (1 pass), "wsplit" (2)
QK_PROJ_MODE = "wsplit"


def _build_core_kernel(DX, N, DQ, DKH, qk_mode=QK_PROJ_MODE, bench=False,
                       bench_reps=0):
    import concourse.bass as bass
    import concourse.mybir as mybir
    import concourse.tile as tile
    from concourse import bacc

    f32 = mybir.dt.float32
    f32r = mybir.dt.float32r

    P = 128
    DT = DX // P            # d-tiles (projection contraction)
    NT512 = N // 512        # n chunks of 512
    NT128 = N // P          # n tiles of 128
    QC = (DQ + 511) // 512  # q chunks of <=512
    QCS = min(DQ, 512)
    KT = DKH // P           # k tiles of 128
    QT128 = DQ // P         # q tiles (out partitions)
    scale = 1.0 / math.sqrt(DQ)

    assert DX % P == 0 and N % 512 == 0 and DQ % P == 0 and DKH % P == 0
    assert DQ % QCS == 0

    nc = bacc.Bacc(None, target_bir_lowering=False, debug=False)

    kind_big = "Internal" if bench else "ExternalInput"
    kind_out = "Internal" if bench else "ExternalOutput"
    xb = nc.dram_tensor("xb", [DX, N], f32, kind=kind_big)
    wqt = nc.dram_tensor("wqt", [DX, DQ], f32, kind=kind_big)
    wkt = nc.dram_tensor("wkt", [DX, DKH], f32, kind=kind_big)
    wvt = nc.dram_tensor("wvt", [DX, DKH], f32, kind=kind_big)
    # tiny input consumed into one output element (value 0 at rest): lets a
    # benchmark chain data dependencies between repeated NEFF executions
    seed = nc.dram_tensor("seed", [1, 1], f32, kind="ExternalInput")
    out = nc.dram_tensor("out", [DQ, N], f32, kind=kind_out)
    sink = (nc.dram_tensor("sink", [1, 1], f32, kind="ExternalOutput")
            if bench else None)

    xv = xb.ap().rearrange("(dt p) n -> p dt n", p=P)
    wqv = wqt.ap().rearrange("(dt p) q -> p dt q", p=P)
    wkv = wkt.ap().rearrange("(dt p) k -> p dt k", p=P)
    wvv = wvt.ap().rearrange("(dt p) k -> p dt k", p=P)

    with tile.TileContext(nc) as tc:
        with (
            tc.tile_pool(name="dram", bufs=1, space="DRAM") as dram,
            tc.tile_pool(name="ps", bufs=8, space="PSUM") as ps,
        ):
            # spills grouped by 4 n-tiles: fine-grained cross-phase deps with
            # batched (1MB-class) reload DMAs. Views: row = t*128 + p.
            NG = NT128 // 4
            qh_d = [dram.tile([4 * P, DQ], f32r, name=f"qh_d{i}").rearrange(
                "(t p) q -> p t q", p=P) for i in range(NG)]
            ql_d = [dram.tile([4 * P, DQ], f32r, name=f"ql_d{i}").rearrange(
                "(t p) q -> p t q", p=P) for i in range(NG)]
            kr_d = [dram.tile([4 * P, DKH], f32r, name=f"kr_d{i}").rearrange(
                "(t p) k -> p t k", p=P) for i in range(NG)]

            pvres_cm = tc.tile_pool(name="pvres", bufs=1)
            pvres = pvres_cm.__enter__()
            # V stays SBUF-resident (64KB/partition): no spill round-trip
            v_res = pvres.tile([P, KT, N], f32r, tag="vres", name="v_res")

            rep_cm = tc.For_i(0, bench_reps, 1) if bench_reps else None
            if rep_cm is not None:
                rep_cm.__enter__()

            # ---------------- Phase 0 + 1: projections ----------------
            with (
                tc.tile_pool(name="pw", bufs=1) as pw,
                tc.tile_pool(name="pwstage", bufs=1) as pwstage,
                tc.tile_pool(name="px", bufs=3) as px,
                tc.tile_pool(name="pev", bufs=2) as pev,
            ):
                # --- weight prep: round W to f32r in SBUF ---
                # The host passes Wq/Wk MINUS 0.5 (zero-mean entries): their
                # f32r rounding error is then incoherent over the
                # d-contraction, so Q and K projections run single-pass f32r.
                # The exact mean term 0.5*colsum(x)[n] is restored with one
                # K=1 matmul per projection psum; its residual rounding error
                # is constant across q and cancels exactly in the softmax.
                wq_h = pw.tile([P, DT, DQ], f32r, tag="wqh")
                wk_h = pw.tile([P, DT, DKH], f32r, tag="wkh")
                wq_l = wk_l = None
                wv_r = pw.tile([P, DT, DKH], f32r, tag="wvr")

                ones_c = pw.tile([P, 1], f32r, tag="ones")
                half_row = pw.tile([1, 512], f32r, tag="half")
                cstage = pwstage.tile([P, 512], f32, tag="wtmp2")
                nc.gpsimd.memset(cstage[:], 1.0)
                nc.vector.tensor_copy(ones_c[:], cstage[:, 0:1])
                nc.gpsimd.memset(cstage[:, 0:512], 0.5)
                nc.vector.tensor_copy(half_row[:], cstage[0:1, 0:512])

                pre_xc = px.tile([P, DT, 512], f32, tag="x", name="xc0")
                nc.sync.dma_start(pre_xc[:], xv[:, :, bass.ds(0, 512)])

                for dt in range(DT):
                    d1 = bass.ds(dt, 1)
                    wtmp = pwstage.tile([P, 1, DQ], f32, tag="wtmp")
                    nc.sync.dma_start(wtmp[:], wqv[:, d1])
                    nc.vector.tensor_copy(wq_h[:, d1], wtmp[:])
                    if dt % 2 == 0:
                        d2 = bass.ds(dt, 2)
                        wtmp2 = pwstage.tile([P, 2, DKH], f32, tag="wtmp2")
                        nc.sync.dma_start(wtmp2[:], wkv[:, d2])
                        nc.vector.tensor_copy(wk_h[:, d2], wtmp2[:])

                        wtmp3 = pwstage.tile([P, 2, DKH], f32, tag="wtmp3")
                        nc.sync.dma_start(wtmp3[:], wvv[:, d2])
                        nc.vector.tensor_copy(wv_r[:, d2], wtmp3[:])

                # --- x chunks: project ---
                for c in range(NT512):
                    ncol = bass.ds(c * 512, 512)
                    if c == 0:
                        xc = pre_xc
                    else:
                        xc = px.tile([P, DT, 512], f32, tag="x", name=f"xc{c}")
                        nc.sync.dma_start(xc[:], xv[:, :, ncol])
                    xr = px.tile([P, DT, 512], f32r, tag="x", name=f"xr{c}")
                    nc.vector.tensor_copy(xr[:], xc[:])

                    # s[n] = colsum_d x (from xr): ones-matmul, [1, 512]
                    sps = ps.tile([P, 512], f32, tag="ps", name=f"sps_c{c}")
                    for dt in range(DT):
                        nc.tensor.matmul(sps[0:1, :], ones_c[:], xr[:, dt],
                                         start=(dt == 0), stop=(dt == DT - 1))
                    s_sb = pev.tile([1, 512], f32r, tag="s_sb")
                    nc.vector.tensor_copy(s_sb[:], sps[0:1, :])

                    # V projection: psum [v-128, n-512]
                    for vt in range(KT):
                        vps = ps.tile([P, 512], f32, tag="ps", name=f"vps{c}_{vt}")
                        vsl = bass.ds(vt * P, P)
                        for dt in range(DT):
                            nc.tensor.matmul(
                                vps[:], wv_r[:, dt, vsl], xr[:, dt],
                                start=(dt == 0), stop=(dt == DT - 1),
                            )
                        nc.vector.tensor_copy(v_res[:, vt, ncol], vps[:])

                    # QT / KT projections per n-subtile
                    for nt in range(4):
                        gnt = c * 4 + nt   # global n-128 tile
                        xsl = bass.ds(nt * P, P)
                        for qc in range(QC):
                            qsl = bass.ds(qc * QCS, QCS)
                            qps = ps.tile([P, QCS], f32, tag="ps",
                                          name=f"qps{gnt}_{qc}")
                            for dt in range(DT):
                                nc.tensor.matmul(
                                    qps[:], xr[:, dt, xsl], wq_h[:, dt, qsl],
                                    start=(dt == 0), stop=False)
                            nc.tensor.matmul(
                                qps[:], s_sb[0:1, xsl], half_row[:, :QCS],
                                start=False, stop=True)
                            qh = pev.tile([P, QCS], f32r, tag="qh")
                            ql = pev.tile([P, QCS], f32r, tag="ql")
                            nc.vector.tensor_copy(qh[:], qps[:])
                            nc.vector.tensor_sub(ql[:], qps[:], qh[:])
                            nc.scalar.dma_start(qh_d[gnt // 4][:, gnt % 4, qsl], qh[:])
                            nc.scalar.dma_start(ql_d[gnt // 4][:, gnt % 4, qsl], ql[:])

                        kps = ps.tile([P, DKH], f32, tag="ps", name=f"kps{gnt}")
                        for dt in range(DT):
                            nc.tensor.matmul(
                                kps[:], xr[:, dt, xsl], wk_h[:, dt],
                                start=(dt == 0), stop=False)
                        nc.tensor.matmul(
                            kps[:], s_sb[0:1, xsl], half_row[:, :DKH],
                            start=False, stop=True)
                        kr = pev.tile([P, DKH], f32r, tag="kr")
                        nc.vector.tensor_copy(kr[:], kps[:])
                        nc.scalar.dma_start(kr_d[gnt // 4][:, gnt % 4], kr[:])

            # ---------------- Phase 2: scoresT + softmax ----------------
            pres_cm = tc.tile_pool(name="resident", bufs=1)
            pres = pres_cm.__enter__()
            scores_sb = [pres.tile([P, DQ], f32, tag=f"sc{kt}", name=f"scores{kt}")
                         for kt in range(KT)]
            p_r = [pres.tile([P, DQ], f32r, tag=f"pr{kt}", name=f"p{kt}")
                   for kt in range(KT)]

            with (
                tc.tile_pool(name="pstream", bufs=3) as pstream,
                tc.tile_pool(name="psmx", bufs=2) as psmx,
                tc.tile_pool(name="pstat", bufs=2) as pstat,
            ):
                for qc in range(QC):
                    qsl = bass.ds(qc * QCS, QCS)
                    s_ps = [ps.tile([P, QCS], f32, tag="ps", name=f"sps{qc}_{i}")
                            for i in range(KT)]
                    for g in range(NG):
                        qh_s = pstream.tile([P, 4, QCS], f32r, tag="qh_s")
                        ql_s = pstream.tile([P, 4, QCS], f32r, tag="ql_s")
                        kr_s = pstream.tile([P, 4, DKH], f32r, tag="kr_s")
                        nc.sync.dma_start(qh_s[:], qh_d[g][:, :, qsl])
                        nc.sync.dma_start(ql_s[:], ql_d[g][:, :, qsl])
                        nc.sync.dma_start(kr_s[:], kr_d[g][:])
                        # in-place re-round after DMA (verifier: f32r matmul
                        # operands need a rounding compute producer)
                        nc.vector.tensor_copy(qh_s[:], qh_s[:])
                        nc.vector.tensor_copy(ql_s[:], ql_s[:])
                        nc.vector.tensor_copy(kr_s[:], kr_s[:])
                        for t in range(4):
                            nt = g * 4 + t
                            for kt in range(KT):
                                ksl = bass.ds(kt * P, P)
                                nc.tensor.matmul(
                                    s_ps[kt][:], kr_s[:, t, ksl], qh_s[:, t],
                                    start=(nt == 0), stop=False)
                                nc.tensor.matmul(
                                    s_ps[kt][:], kr_s[:, t, ksl], ql_s[:, t],
                                    start=False, stop=(nt == NT128 - 1))
                    for kt in range(KT):
                        nc.vector.tensor_copy(scores_sb[kt][:, qsl], s_ps[kt][:])

                # softmax over q (free axis) per k row; fold in 1/sqrt(DQ)
                for kt in range(KT):
                    m = pstat.tile([P, 1], f32, tag="m")
                    negm = pstat.tile([P, 1], f32, tag="negm")
                    den = pstat.tile([P, 1], f32, tag="den")
                    rden = pstat.tile([P, 1], f32, tag="rden")
                    nc.vector.reduce_max(m[:], scores_sb[kt][:],
                                         axis=mybir.AxisListType.X)
                    nc.vector.tensor_scalar_mul(negm[:], m[:], -scale)
                    e = psmx.tile([P, DQ], f32, tag="e")
                    nc.scalar.activation(
                        e[:], scores_sb[kt][:],
                        mybir.ActivationFunctionType.Exp,
                        bias=negm[:], scale=scale, accum_out=den[:])
                    nc.vector.reciprocal(rden[:], den[:])
                    nc.vector.tensor_scalar_mul(p_r[kt][:], e[:], rden[:])

            # ---------------- Phase 3: out = pT.T @ V ----------------
            with (
                tc.tile_pool(name="pout", bufs=4) as pout,
                tc.tile_pool(name="pseed", bufs=1) as pseed,
            ):
                seed_sb = pseed.tile([1, 1], f32, tag="seed")
                nc.sync.dma_start(seed_sb[:], seed.ap())
                outv = out.ap().rearrange("(qt p) n -> p qt n", p=P)
                for c in range(NT512):
                    ncol = bass.ds(c * 512, 512)
                    for qg in range(QT128 // 4):
                        osb = pout.tile([P, 4, 512], f32, tag="osb")
                        for qi in range(4):
                            qt = qg * 4 + qi
                            ops = ps.tile([P, 512], f32, tag="ps",
                                          name=f"ops{c}_{qt}")
                            qsl2 = bass.ds(qt * P, P)
                            for kt in range(KT):
                                nc.tensor.matmul(
                                    ops[:], p_r[kt][:, qsl2],
                                    v_res[:, kt, ncol],
                                    start=(kt == 0), stop=(kt == KT - 1))
                            nc.vector.tensor_copy(osb[:, qi], ops[:])
                            if c == 0 and qt == 0:
                                nc.vector.tensor_scalar_add(
                                    osb[0:1, 0, 0:1], ops[0:1, 0:1], seed_sb[:])
                                if sink is not None:
                                    nc.sync.dma_start(sink.ap(), osb[0:1, 0, 0:1])
                        nc.gpsimd.dma_start(
                            outv[:, qg * 4:(qg + 1) * 4, ncol], osb[:])
            pres_cm.__exit__(None, None, None)
            if rep_cm is not None:
                rep_cm.__exit__(None, None, None)
            pvres_cm.__exit__(None, None, None)

    nc.compile()
    return nc


_CACHE = {}


def _get_nc(DX, N, DQ, DKH, qk_mode=QK_PROJ_MODE):
    key = (DX, N, DQ, DKH, qk_mode)
    if key not in _CACHE:
        _CACHE[key] = _build_core_kernel(DX, N, DQ, DKH, qk_mode)
    return _CACHE[key]


def _run(x, Wq, Wk, Wv, **spmd_kwargs):
    """Run the SPMD kernel; returns (out, BassKernelResults)."""
    from concourse.bass_utils import run_bass_kernel_spmd

    B, DX, N = x.shape
    DQ = Wq.shape[0]
    DK = Wk.shape[0]
    assert (B, DX, N, DQ, DK) == (B_FULL, DX_FULL, N_FULL, DQ_FULL, DK_FULL)
    DKH = DK // 2

    nc = _get_nc(DX, N, DQ, DKH)

    # Wq/Wk are shipped mean-removed (entries - 0.5); the kernel restores
    # the exact 0.5*colsum(x) term on-chip (see builder comment)
    WqT = np.ascontiguousarray(Wq.T, dtype=np.float32) - np.float32(0.5)
    WkT = np.ascontiguousarray(Wk.T, dtype=np.float32) - np.float32(0.5)
    WvT = np.ascontiguousarray(Wv.T, dtype=np.float32)

    in_maps = []
    for c in range(N_CORES):
        b, h = divmod(c, 2)
        hsl = slice(h * DKH, (h + 1) * DKH)
        in_maps.append({
            "xb": np.ascontiguousarray(x[b], dtype=np.float32),
            "wqt": WqT,
            "wkt": np.ascontiguousarray(WkT[:, hsl]),
            "wvt": np.ascontiguousarray(WvT[:, hsl]),
            "seed": np.zeros((1, 1), np.float32),
        })

    res = run_bass_kernel_spmd(nc, in_maps, core_ids=list(range(N_CORES)),
                               **spmd_kwargs)
    out = np.empty((B, DQ, N), np.float32)
    for b in range(B):
        out[b] = res.results[2 * b]["out"] + res.results[2 * b + 1]["out"]
    return out, res


def kernel(x, Wq, Wk, Wv):
    return _run(x, Wq, Wk, Wv)[0]



# revision 6
# speedup vs baseline: 1.8032x; 1.8032x over previous
"""TRN2 Bass kernel for nn_Attention_369367187796 (fused single-sweep).

Reference (B=4, DX=1024, N=4096, DQ=DK=DV=1024, fp32):
    Q = Wq @ x[b]; K = Wk @ x[b]; V = Wv @ x[b]
    scores = Q @ K.T   (contract n)
    p = softmax(scores / sqrt(DQ), axis=q)     <- softmax over q!
    out[q,n] = sum_k p[q,k] V[k,n]

Sharding: 8 cores = 4 batches x 2 k-halves. Each core: full Q, its half of
K and V, scoresT[k_half, q] (softmax over q = free axis, fully local), and
the partial out[q,n] over its k-half. Host sums the two partials per batch.

vs the previous (spill-based, 2-pass) kernel, this version:
  - fuses projections + scores into one sweep over 256-col n-chunks:
    QT/KT tiles live only per-chunk in SBUF, scoresT accumulates in PSUM
    per chunk and is flushed (vector add) into an SBUF f32 accumulator.
    No QT/KT DRAM spill/reload (saved ~90MB of DMA traffic).
  - single-pass f32r scores matmul. Numpy-exact simulation of the f32r
    rounding on the real (deterministic) inputs: 1-pass 6.1e-4 vs 2-pass
    5.5e-4 rel err — the hi/lo Q split bought nothing vs the 2e-2 gate.
  - drops the Q mean-restore entirely: Wq is shipped mean-removed
    (entries - 0.5) and the restore term 0.5*colsum(x)[n] is constant
    across q for fixed k after the scores contraction, so it cancels
    exactly in the softmax (verified in simulation).
  - K mean-restore via a host-computed 0.5*colsum(x) shipped as input
    "sh" [128, NT128] and fused into the KT PSUM eviction as a
    per-partition tensor_scalar_add (no K=1 restore matmuls, no on-chip
    ones-matmul colsum).

Layouts (per core):
  QT[n,q], KT[n,k] computed transposed (psum partitions = n-subtile) so
  scores contracts n on partitions and softmax lands on the free q axis:
    QT[n,q]:      lhsT = xr [d, n-sub], rhs = WqT [d, q]
    scoresT[k,q]: lhsT = KT [n, k],     rhs = QT [n, q]
    out[q,n]:     lhsT = pT [k, q],     rhs = V  [k, n]
  V stays SBUF-resident f32r (64KB/partition).
"""

import math

import numpy as np

B_FULL, DX_FULL, N_FULL = 4, 1024, 4096
DQ_FULL = DK_FULL = 1024
N_CORES = 8


def _build_core_kernel(DX, N, DQ, DKH, bench=False, bench_reps=0):
    import concourse.bass as bass
    import concourse.mybir as mybir
    import concourse.tile as tile
    from concourse import bacc

    f32 = mybir.dt.float32
    f32r = mybir.dt.float32r

    P = 128
    DT = DX // P            # d-tiles (projection contraction)
    CW = 256                # n-chunk width (SBUF-sized)
    NCH = N // CW           # n chunks
    NSUB = CW // P          # n-subtiles per chunk
    NT128 = N // P          # global n tiles of 128
    QC = DQ // 512          # q chunks of 512
    KT = DKH // P           # k tiles of 128
    QT128 = DQ // P         # q tiles (out partitions)
    OCW = 512               # out-phase n-chunk width
    scale = 1.0 / math.sqrt(DQ)

    assert DX % P == 0 and N % CW == 0 and DQ % 512 == 0 and DKH % P == 0

    nc = bacc.Bacc(None, target_bir_lowering=False, debug=False)

    kind_big = "Internal" if bench else "ExternalInput"
    kind_out = "Internal" if bench else "ExternalOutput"
    xb = nc.dram_tensor("xb", [DX, N], f32, kind=kind_big)
    wqt = nc.dram_tensor("wqt", [DX, DQ], f32, kind=kind_big)
    wkt = nc.dram_tensor("wkt", [DX, DKH], f32, kind=kind_big)
    wvt = nc.dram_tensor("wvt", [DX, DKH], f32, kind=kind_big)
    sh = nc.dram_tensor("sh", [P, NT128], f32, kind=kind_big)
    # tiny input consumed into one output element (value 0 at rest): lets a
    # benchmark chain data dependencies between repeated NEFF executions
    seed = nc.dram_tensor("seed", [1, 1], f32, kind="ExternalInput")
    out = nc.dram_tensor("out", [DQ, N], f32, kind=kind_out)
    sink = (nc.dram_tensor("sink", [1, 1], f32, kind="ExternalOutput")
            if bench else None)

    xv = xb.ap().rearrange("(dt p) n -> p dt n", p=P)
    wqv = wqt.ap().rearrange("(dt p) q -> p dt q", p=P)
    wkv = wkt.ap().rearrange("(dt p) k -> p dt k", p=P)
    wvv = wvt.ap().rearrange("(dt p) k -> p dt k", p=P)

    with tile.TileContext(nc) as tc:
        with (
            tc.tile_pool(name="ps", bufs=8, space="PSUM") as ps,
            tc.tile_pool(name="pvres", bufs=1) as pvres,
            tc.tile_pool(name="pscore", bufs=1) as pscore,
        ):
            # V resident f32r (64KB/partition); scoresT f32 accumulators
            v_res = pvres.tile([P, KT, N], f32r, tag="vres", name="v_res")
            scores_sb = [pscore.tile([P, DQ], f32, tag=f"sc{kt}",
                                     name=f"scores{kt}") for kt in range(KT)]

            rep_cm = tc.For_i(0, bench_reps, 1) if bench_reps else None
            if rep_cm is not None:
                rep_cm.__enter__()

            # ---------------- fused sweep: projections + scores ----------
            with (
                tc.tile_pool(name="pw", bufs=1) as pw,
                tc.tile_pool(name="pwstage", bufs=2) as pwstage,
                tc.tile_pool(name="pxc", bufs=2) as pxc,
                tc.tile_pool(name="pxr", bufs=1) as pxr,
                tc.tile_pool(name="pev", bufs=1) as pev,
                tc.tile_pool(name="psh", bufs=1) as psh,
            ):
                # x chunk 0 prefetch first (weights go on the scalar DMA
                # queue so x-chunk streaming isn't starved behind them)
                xcs = [pxc.tile([P, DT, CW], f32, tag="xc", name=f"xc{c}")
                       for c in range(2)]
                nc.sync.dma_start(xcs[0][:], xv[:, :, bass.ds(0, CW)])
                s_sb = psh.tile([P, NT128], f32, tag="s_sb")
                nc.scalar.dma_start(s_sb[:], sh.ap())

                # weights -> SBUF f32r, one tile per d-tile for fine deps
                wq_sb = [pw.tile([P, DQ], f32r, tag=f"wq{dt}", name=f"wq{dt}")
                         for dt in range(DT)]
                wk_sb = [pw.tile([P, DKH], f32r, tag=f"wk{dt}", name=f"wk{dt}")
                         for dt in range(DT)]
                wv_sb = [pw.tile([P, DKH], f32r, tag=f"wv{dt}", name=f"wv{dt}")
                         for dt in range(DT)]
                for dt in range(DT):
                    d1 = bass.ds(dt, 1)
                    wtmp = pwstage.tile([P, 1, DQ], f32, tag="wtmp")
                    nc.scalar.dma_start(wtmp[:], wqv[:, d1])
                    nc.vector.tensor_copy(wq_sb[dt][:], wtmp[:, 0])
                    wtmp2 = pwstage.tile([P, 2, DKH], f32, tag="wtmp2")
                    nc.scalar.dma_start(wtmp2[:, 0:1], wkv[:, d1])
                    nc.scalar.dma_start(wtmp2[:, 1:2], wvv[:, d1])
                    nc.vector.tensor_copy(wk_sb[dt][:], wtmp2[:, 0])
                    nc.vector.tensor_copy(wv_sb[dt][:], wtmp2[:, 1])

                for c in range(NCH):
                    xc = xcs[c % 2]
                    if c + 1 < NCH:
                        nc.sync.dma_start(
                            xcs[(c + 1) % 2][:],
                            xv[:, :, bass.ds((c + 1) * CW, CW)])
                    xr = pxr.tile([P, DT, CW], f32r, tag="xr", name=f"xr{c}")
                    nc.vector.tensor_copy(xr[:], xc[:])

                    qh = pev.tile([P, NSUB, DQ], f32r, tag="qh",
                                  name=f"qh{c}")
                    kr = pev.tile([P, NSUB, DKH], f32r, tag="kr",
                                  name=f"kr{c}")
                    # QT / KT projections per n-subtile (x-subtile is the
                    # stationary operand, shared by 3 streams per d-tile)
                    for nt in range(NSUB):
                        gnt = c * NSUB + nt
                        xsl = bass.ds(nt * P, P)
                        qps = [ps.tile([P, 512], f32, tag="ps",
                                       name=f"qps{gnt}_{qc}")
                               for qc in range(QC)]
                        kps = ps.tile([P, DKH], f32, tag="ps",
                                      name=f"kps{gnt}")
                        for dt in range(DT):
                            st, sp = dt == 0, dt == DT - 1
                            for qc in range(QC):
                                nc.tensor.matmul(
                                    qps[qc][:], xr[:, dt, xsl],
                                    wq_sb[dt][:, bass.ds(qc * 512, 512)],
                                    start=st, stop=sp)
                            nc.tensor.matmul(kps[:], xr[:, dt, xsl],
                                             wk_sb[dt][:], start=st, stop=sp)
                        for qc in range(QC):
                            nc.vector.tensor_copy(
                                qh[:, nt, bass.ds(qc * 512, 512)],
                                qps[qc][:])
                        # K mean-restore fused into eviction: +0.5*colsum(x)
                        nc.vector.tensor_scalar_add(
                            kr[:, nt], kps[:], s_sb[:, bass.ds(gnt, 1)])

                    # V projection: psum [v-128, n-chunk]
                    for vt in range(KT):
                        vps = ps.tile([P, CW], f32, tag="ps",
                                      name=f"vps{c}_{vt}")
                        vsl = bass.ds(vt * P, P)
                        for dt in range(DT):
                            nc.tensor.matmul(
                                vps[:], wv_sb[dt][:, vsl], xr[:, dt],
                                start=(dt == 0), stop=(dt == DT - 1))
                        nc.gpsimd.tensor_copy(
                            v_res[:, vt, bass.ds(c * CW, CW)], vps[:])

                    # scoresT += KT.T @ QT for this chunk (PSUM accumulate
                    # over the chunk's n-subtiles, then flush to SBUF on the
                    # Pool engine). kt-outer: one kr weight load serves both
                    # q-chunk streams, and kt0's flush lands early so the
                    # softmax chain can start during the last chunk's tail.
                    for kt in range(KT):
                        ksl = bass.ds(kt * P, P)
                        s_ps = [ps.tile([P, 512], f32, tag="ps",
                                        name=f"sps{c}_{kt}_{qc}")
                                for qc in range(QC)]
                        for nt in range(NSUB):
                            for qc in range(QC):
                                nc.tensor.matmul(
                                    s_ps[qc][:], kr[:, nt, ksl],
                                    qh[:, nt, bass.ds(qc * 512, 512)],
                                    start=(nt == 0), stop=(nt == NSUB - 1))
                        for qc in range(QC):
                            qsl = bass.ds(qc * 512, 512)
                            if c == 0:
                                nc.gpsimd.tensor_copy(
                                    scores_sb[kt][:, qsl], s_ps[qc][:])
                            else:
                                nc.gpsimd.tensor_add(
                                    scores_sb[kt][:, qsl],
                                    scores_sb[kt][:, qsl], s_ps[qc][:])

            # ---------------- softmax over q + out = pT.T @ V -------------
            with (
                tc.tile_pool(name="ppr", bufs=1) as ppr,
                tc.tile_pool(name="psmx", bufs=2) as psmx,
                tc.tile_pool(name="pstat", bufs=2) as pstat,
                tc.tile_pool(name="pout", bufs=4) as pout,
                tc.tile_pool(name="pseed", bufs=1) as pseed,
            ):
                p_r = [ppr.tile([P, DQ], f32r, tag=f"pr{kt}", name=f"p{kt}")
                       for kt in range(KT)]
                for kt in range(KT):
                    m = pstat.tile([P, 1], f32, tag="m")
                    negm = pstat.tile([P, 1], f32, tag="negm")
                    den = pstat.tile([P, 1], f32, tag="den")
                    rden = pstat.tile([P, 1], f32, tag="rden")
                    nc.vector.reduce_max(m[:], scores_sb[kt][:],
                                         axis=mybir.AxisListType.X)
                    nc.vector.tensor_scalar_mul(negm[:], m[:], -scale)
                    e = psmx.tile([P, DQ], f32, tag="e")
                    nc.scalar.activation(
                        e[:], scores_sb[kt][:],
                        mybir.ActivationFunctionType.Exp,
                        bias=negm[:], scale=scale, accum_out=den[:])
                    nc.vector.reciprocal(rden[:], den[:])
                    nc.vector.tensor_scalar_mul(p_r[kt][:], e[:], rden[:])

                seed_sb = pseed.tile([1, 1], f32, tag="seed")
                nc.sync.dma_start(seed_sb[:], seed.ap())
                outv = out.ap().rearrange("(qt p) n -> p qt n", p=P)
                NOC = N // OCW
                # kt-outer across all 8 psum banks: the first chunk's kt0
                # matmuls start as soon as p_r[0] lands, overlapping the
                # softmax chain of kt1..3
                for c in range(NOC):
                    ncol = bass.ds(c * OCW, OCW)
                    ops = [ps.tile([P, OCW], f32, tag="ps",
                                   name=f"ops{c}_{qt}")
                           for qt in range(QT128)]
                    for kt in range(KT):
                        for qt in range(QT128):
                            nc.tensor.matmul(
                                ops[qt][:], p_r[kt][:, bass.ds(qt * P, P)],
                                v_res[:, kt, ncol],
                                start=(kt == 0), stop=(kt == KT - 1))
                    for qg in range(QT128 // 4):
                        osb = pout.tile([P, 4, OCW], f32, tag="osb")
                        for qi in range(4):
                            qt = qg * 4 + qi
                            nc.vector.tensor_copy(osb[:, qi], ops[qt][:])
                            if c == 0 and qt == 0:
                                nc.vector.tensor_scalar_add(
                                    osb[0:1, 0, 0:1], ops[0][0:1, 0:1],
                                    seed_sb[:])
                                if sink is not None:
                                    nc.sync.dma_start(sink.ap(),
                                                      osb[0:1, 0, 0:1])
                        if c == NOC - 1:
                            # fine-grained tail: don't serialize the final
                            # 1MB store behind the last eviction
                            for qi in range(4):
                                nc.gpsimd.dma_start(
                                    outv[:, qg * 4 + qi:qg * 4 + qi + 1,
                                         ncol],
                                    osb[:, qi:qi + 1])
                        else:
                            nc.gpsimd.dma_start(
                                outv[:, qg * 4:(qg + 1) * 4, ncol], osb[:])

            if rep_cm is not None:
                rep_cm.__exit__(None, None, None)

    nc.compile()
    return nc


_CACHE = {}


def _get_nc(DX, N, DQ, DKH):
    key = (DX, N, DQ, DKH)
    if key not in _CACHE:
        _CACHE[key] = _build_core_kernel(DX, N, DQ, DKH)
    return _CACHE[key]


def _run(x, Wq, Wk, Wv, **spmd_kwargs):
    """Run the SPMD kernel; returns (out, BassKernelResults)."""
    from concourse.bass_utils import run_bass_kernel_spmd

    B, DX, N = x.shape
    DQ = Wq.shape[0]
    DK = Wk.shape[0]
    assert (B, DX, N, DQ, DK) == (B_FULL, DX_FULL, N_FULL, DQ_FULL, DK_FULL)
    DKH = DK // 2
    P = 128

    nc = _get_nc(DX, N, DQ, DKH)

    # Wq/Wk shipped mean-removed (entries - 0.5): zero-mean f32r rounding.
    # Q needs no restore (cancels in softmax); K's restore is the shipped
    # host-exact sh = 0.5*colsum(x), laid out [p, ntile] for the kernel.
    WqT = np.ascontiguousarray(Wq.T, dtype=np.float32) - np.float32(0.5)
    WkT = np.ascontiguousarray(Wk.T, dtype=np.float32) - np.float32(0.5)
    WvT = np.ascontiguousarray(Wv.T, dtype=np.float32)

    in_maps = []
    for c in range(N_CORES):
        b, h = divmod(c, 2)
        hsl = slice(h * DKH, (h + 1) * DKH)
        s = (0.5 * x[b].astype(np.float32).sum(axis=0)).astype(np.float32)
        in_maps.append({
            "xb": np.ascontiguousarray(x[b], dtype=np.float32),
            "wqt": WqT,
            "wkt": np.ascontiguousarray(WkT[:, hsl]),
            "wvt": np.ascontiguousarray(WvT[:, hsl]),
            "sh": np.ascontiguousarray(s.reshape(N // P, P).T),
            "seed": np.zeros((1, 1), np.float32),
        })

    res = run_bass_kernel_spmd(nc, in_maps, core_ids=list(range(N_CORES)),
                               **spmd_kwargs)
    out = np.empty((B, DQ, N), np.float32)
    for b in range(B):
        out[b] = res.results[2 * b]["out"] + res.results[2 * b + 1]["out"]
    return out, res


def kernel(x, Wq, Wk, Wv):
    return _run(x, Wq, Wk, Wv)[0]


# revision 11
# speedup vs baseline: 2.3413x; 1.2984x over previous
"""TRN2 Bass kernel for nn_Attention_369367187796 (fused single-sweep).

Reference (B=4, DX=1024, N=4096, DQ=DK=DV=1024, fp32):
    Q = Wq @ x[b]; K = Wk @ x[b]; V = Wv @ x[b]
    scores = Q @ K.T   (contract n)
    p = softmax(scores / sqrt(DQ), axis=q)     <- softmax over q!
    out[q,n] = sum_k p[q,k] V[k,n]

Sharding: 8 cores = 4 batches x 2 k-halves. Each core: full Q, its half of
K and V, scoresT[k_half, q] (softmax over q = free axis, fully local), and
the partial out[q,n] over its k-half. Host sums the two partials per batch.

Design (vs the original spill-based 2-pass kernel, 538us -> ~350us):
  - fused single sweep over 512-col n-chunks: QT/KT tiles live only
    per-chunk in SBUF, scoresT accumulates in PSUM per chunk and is
    flushed (Pool-engine add) into an SBUF f32 accumulator. No QT/KT
    DRAM spill/reload.
  - single-pass f32r scores matmul. Numpy-exact rounding simulation on
    the real (deterministic) inputs: 1-pass 6.1e-4 vs 2-pass 5.5e-4 rel
    err (gate is 2e-2); HW measured 6.06e-4, matching the sim.
  - x and the weights ship as bf16 (halves input DMA to 14.7MB, removes
    all staging copies; bf16 operands need no f32r-producer op).
    Simulated end-to-end rel err with bf16 x+W: 8.4e-3.
  - Q mean-restore dropped entirely: Wq ships mean-removed (entries -
    0.5); the restore term is constant across q for fixed k after the
    scores contraction, so it cancels exactly in the softmax.
  - K mean-restore = host-computed 0.5*colsum(x) ("sh" input), fused
    into the KT PSUM eviction as a per-partition tensor_scalar_add.
  - softmax scalar chain (negm/exp/recip) runs back-to-back on the ACT
    queue; out-phase matmuls are kt-outer on the first n-chunk so they
    start as soon as p_r[0] lands.

Layouts (per core):
  QT[n,q], KT[n,k] computed transposed (psum partitions = n-subtile) so
  scores contracts n on partitions and softmax lands on the free q axis:
    QT[n,q]:      lhsT = x [d, n-sub] (bf16), rhs = WqT [d, q] (bf16)
    scoresT[k,q]: lhsT = KT [n, k] (f32r),    rhs = QT [n, q] (f32r)
    out[q,n]:     lhsT = pT [k, q] (f32r),    rhs = V  [k, n] (f32r)
  V stays SBUF-resident f32r (64KB/partition).
"""

import math

import numpy as np

B_FULL, DX_FULL, N_FULL = 4, 1024, 4096
DQ_FULL = DK_FULL = 1024
N_CORES = 8


def _build_core_kernel(DX, N, DQ, DKH, bench=False, bench_reps=0):
    import concourse.bass as bass
    import concourse.mybir as mybir
    import concourse.tile as tile
    from concourse import bacc

    f32 = mybir.dt.float32
    f32r = mybir.dt.float32r
    bf16 = mybir.dt.bfloat16

    P = 128
    DT = DX // P            # d-tiles (projection contraction)
    CW = 512                # n-chunk width
    NCH = N // CW           # n chunks
    NSUB = CW // P          # n-subtiles per chunk
    NT128 = N // P          # global n tiles of 128
    QC = DQ // 512          # q chunks of 512
    KT = DKH // P           # k tiles of 128
    QT128 = DQ // P         # q tiles (out partitions)
    OCW = 512               # out-phase n-chunk width
    scale = 1.0 / math.sqrt(DQ)

    assert DX % P == 0 and N % CW == 0 and DQ % 512 == 0 and DKH % P == 0

    nc = bacc.Bacc(None, target_bir_lowering=False, debug=False)

    kind_big = "Internal" if bench else "ExternalInput"
    kind_out = "Internal" if bench else "ExternalOutput"
    xb = nc.dram_tensor("xb", [DX, N], bf16, kind=kind_big)
    wqt = nc.dram_tensor("wqt", [DX, DQ], bf16, kind=kind_big)
    wkt = nc.dram_tensor("wkt", [DX, DKH], bf16, kind=kind_big)
    wvt = nc.dram_tensor("wvt", [DX, DKH], bf16, kind=kind_big)
    sh = nc.dram_tensor("sh", [P, NT128], f32, kind=kind_big)
    # tiny input consumed into one output element (value 0 at rest): lets a
    # benchmark chain data dependencies between repeated NEFF executions
    seed = nc.dram_tensor("seed", [1, 1], f32, kind="ExternalInput")
    out = nc.dram_tensor("out", [DQ, N], f32, kind=kind_out)
    sink = (nc.dram_tensor("sink", [1, 1], f32, kind="ExternalOutput")
            if bench else None)

    xv = xb.ap().rearrange("(dt p) n -> p dt n", p=P)
    wqv = wqt.ap().rearrange("(dt p) q -> p dt q", p=P)
    wkv = wkt.ap().rearrange("(dt p) k -> p dt k", p=P)
    wvv = wvt.ap().rearrange("(dt p) k -> p dt k", p=P)

    with tile.TileContext(nc) as tc:
        with (
            tc.tile_pool(name="ps", bufs=8, space="PSUM") as ps,
            tc.tile_pool(name="pvres", bufs=1) as pvres,
            tc.tile_pool(name="pscore", bufs=1) as pscore,
            tc.tile_pool(name="ppr", bufs=1) as ppr,
            tc.tile_pool(name="psmx", bufs=2) as psmx,
            tc.tile_pool(name="pstat", bufs=2) as pstat,
        ):
            # V resident f32r (64KB/partition); scoresT f32 accumulators
            v_res = pvres.tile([P, KT, N], f32r, tag="vres", name="v_res")
            scores_sb = [pscore.tile([P, DQ], f32, tag=f"sc{kt}",
                                     name=f"scores{kt}") for kt in range(KT)]
            p_r = [ppr.tile([P, DQ], f32r, tag=f"pr{kt}", name=f"p{kt}")
                   for kt in range(KT)]

            def softmax_kt(kt):
                # negm/exp run back-to-back on the ACT queue
                m = pstat.tile([P, 1], f32, tag="m", name=f"m{kt}")
                negm = pstat.tile([P, 1], f32, tag="negm", name=f"negm{kt}")
                den = pstat.tile([P, 1], f32, tag="den", name=f"den{kt}")
                rden = pstat.tile([P, 1], f32, tag="rden", name=f"rden{kt}")
                nc.vector.reduce_max(m[:], scores_sb[kt][:],
                                     axis=mybir.AxisListType.X)
                nc.scalar.activation(
                    negm[:], m[:], mybir.ActivationFunctionType.Copy,
                    scale=-scale)
                e = psmx.tile([P, DQ], f32, tag="e", name=f"e{kt}")
                nc.scalar.activation(
                    e[:], scores_sb[kt][:],
                    mybir.ActivationFunctionType.Exp,
                    bias=negm[:], scale=scale, accum_out=den[:])
                nc.vector.reciprocal(rden[:], den[:])
                nc.vector.tensor_scalar_mul(p_r[kt][:], e[:], rden[:])

            rep_cm = tc.For_i(0, bench_reps, 1) if bench_reps else None
            if rep_cm is not None:
                rep_cm.__enter__()

            # ---------------- fused sweep: projections + scores ----------
            with (
                tc.tile_pool(name="pw", bufs=1) as pw,
                tc.tile_pool(name="pxc", bufs=2) as pxc,
                tc.tile_pool(name="pev", bufs=1) as pev,
                tc.tile_pool(name="psh", bufs=1) as psh,
            ):
                # x chunk 0 first; weights (bf16, no staging) next; the
                # x stream and weights share DMA bandwidth but the bf16
                # halving keeps the early chunks fed
                xcs = [pxc.tile([P, DT, CW], bf16, tag="xc", name=f"xc{c}")
                       for c in range(2)]
                # chunk 0 in halves: the first projections need only the
                # first n-subtiles, so the PE starts ~3us earlier
                nc.sync.dma_start(xcs[0][:, :, bass.ds(0, CW // 2)],
                                  xv[:, :, bass.ds(0, CW // 2)])
                nc.sync.dma_start(xcs[0][:, :, bass.ds(CW // 2, CW // 2)],
                                  xv[:, :, bass.ds(CW // 2, CW // 2)])
                s_sb = psh.tile([P, NT128], f32, tag="s_sb")
                nc.scalar.dma_start(s_sb[:], sh.ap())

                wq_sb = [pw.tile([P, 1, DQ], bf16, tag=f"wq{dt}",
                                 name=f"wq{dt}") for dt in range(DT)]
                wk_sb = [pw.tile([P, 1, DKH], bf16, tag=f"wk{dt}",
                                 name=f"wk{dt}") for dt in range(DT)]
                wv_sb = [pw.tile([P, 1, DKH], bf16, tag=f"wv{dt}",
                                 name=f"wv{dt}") for dt in range(DT)]
                for dt in range(DT):
                    d1 = bass.ds(dt, 1)
                    nc.scalar.dma_start(wq_sb[dt][:], wqv[:, d1])
                    nc.scalar.dma_start(wk_sb[dt][:], wkv[:, d1])
                    nc.scalar.dma_start(wv_sb[dt][:], wvv[:, d1])
                # chunk-1 prefetch from the ACT queue so it issues after the
                # weights (it has no buffer-reuse dependency to pace it);
                # later prefetches are paced by the WAR dep on the xc buffer
                nc.scalar.dma_start(xcs[1][:], xv[:, :, bass.ds(CW, CW)])

                for c in range(NCH):
                    xc = xcs[c % 2]
                    if c >= 1 and c + 1 < NCH:
                        nc.sync.dma_start(
                            xcs[(c + 1) % 2][:],
                            xv[:, :, bass.ds((c + 1) * CW, CW)])
                    qh = pev.tile([P, NSUB, DQ], f32r, tag="qh",
                                  name=f"qh{c}")
                    kr = pev.tile([P, NSUB, DKH], f32r, tag="kr",
                                  name=f"kr{c}")
                    # QT / KT projections per n-subtile (x-subtile is the
                    # stationary operand, shared by 3 streams per d-tile)
                    for nt in range(NSUB):
                        gnt = c * NSUB + nt
                        xsl = bass.ds(nt * P, P)
                        qps = [ps.tile([P, 512], f32, tag="ps",
                                       name=f"qps{gnt}_{qc}")
                               for qc in range(QC)]
                        kps = ps.tile([P, DKH], f32, tag="ps",
                                      name=f"kps{gnt}")
                        for dt in range(DT):
                            st, sp = dt == 0, dt == DT - 1
                            for qc in range(QC):
                                nc.tensor.matmul(
                                    qps[qc][:], xc[:, dt, xsl],
                                    wq_sb[dt][:, 0, bass.ds(qc * 512, 512)],
                                    start=st, stop=sp)
                            nc.tensor.matmul(kps[:], xc[:, dt, xsl],
                                             wk_sb[dt][:, 0], start=st,
                                             stop=sp)
                        for qc in range(QC):
                            nc.vector.tensor_copy(
                                qh[:, nt, bass.ds(qc * 512, 512)],
                                qps[qc][:])
                        # K mean-restore fused into eviction: +0.5*colsum(x)
                        nc.vector.tensor_scalar_add(
                            kr[:, nt], kps[:], s_sb[:, bass.ds(gnt, 1)])

                    # V projection: psum [v-128, n-chunk]
                    for vt in range(KT):
                        vps = ps.tile([P, CW], f32, tag="ps",
                                      name=f"vps{c}_{vt}")
                        vsl = bass.ds(vt * P, P)
                        for dt in range(DT):
                            nc.tensor.matmul(
                                vps[:], wv_sb[dt][:, 0, vsl], xc[:, dt],
                                start=(dt == 0), stop=(dt == DT - 1))
                        nc.scalar.activation(
                            v_res[:, vt, bass.ds(c * CW, CW)], vps[:],
                            mybir.ActivationFunctionType.Copy)

                    # scoresT += KT.T @ QT for this chunk (PSUM accumulate
                    # over the chunk's n-subtiles, then flush to SBUF on the
                    # Pool engine). kt-outer: one kr weight load serves both
                    # q-chunk streams, and kt0's flush lands early so the
                    # softmax chain can start during the last chunk's tail.
                    for kt in range(KT):
                        ksl = bass.ds(kt * P, P)
                        s_ps = [ps.tile([P, 512], f32, tag="ps",
                                        name=f"sps{c}_{kt}_{qc}")
                                for qc in range(QC)]
                        for nt in range(NSUB):
                            for qc in range(QC):
                                nc.tensor.matmul(
                                    s_ps[qc][:], kr[:, nt, ksl],
                                    qh[:, nt, bass.ds(qc * 512, 512)],
                                    start=(nt == 0), stop=(nt == NSUB - 1))
                        for qc in range(QC):
                            qsl = bass.ds(qc * 512, 512)
                            if c == 0:
                                nc.vector.tensor_copy(
                                    scores_sb[kt][:, qsl], s_ps[qc][:])
                            else:
                                nc.vector.tensor_add(
                                    scores_sb[kt][:, qsl],
                                    scores_sb[kt][:, qsl], s_ps[qc][:])
                        if c == NCH - 1:
                            softmax_kt(kt)

            # ---------------- softmax over q + out = pT.T @ V -------------
            with (
                tc.tile_pool(name="pout", bufs=4) as pout,
                tc.tile_pool(name="pseed", bufs=1) as pseed,
            ):
                seed_sb = pseed.tile([1, 1], f32, tag="seed")
                nc.sync.dma_start(seed_sb[:], seed.ap())
                outv = out.ap().rearrange("(qt p) n -> p qt n", p=P)
                NOC = N // OCW
                for c in range(NOC):
                    ncol = bass.ds(c * OCW, OCW)
                    if c == 0:
                        # kt-outer across all 8 psum banks: kt0 matmuls
                        # start as soon as p_r[0] lands, overlapping the
                        # softmax chain of kt1..3
                        ops = [ps.tile([P, OCW], f32, tag="ps",
                                       name=f"ops{c}_{qt}")
                               for qt in range(QT128)]
                        for kt in range(KT):
                            for qt in range(QT128):
                                nc.tensor.matmul(
                                    ops[qt][:],
                                    p_r[kt][:, bass.ds(qt * P, P)],
                                    v_res[:, kt, ncol],
                                    start=(kt == 0), stop=(kt == KT - 1))
                        for qg in range(QT128 // 4):
                            osb = pout.tile([P, 4, OCW], f32, tag="osb")
                            for qi in range(4):
                                qt = qg * 4 + qi
                                nc.vector.tensor_copy(osb[:, qi],
                                                      ops[qt][:])
                                if qt == 0:
                                    nc.vector.tensor_scalar_add(
                                        osb[0:1, 0, 0:1], ops[0][0:1, 0:1],
                                        seed_sb[:])
                                    if sink is not None:
                                        nc.sync.dma_start(
                                            sink.ap(), osb[0:1, 0, 0:1])
                            nc.gpsimd.dma_start(
                                outv[:, qg * 4:(qg + 1) * 4, ncol], osb[:])
                    else:
                        # qt-major: evictions and stores trail each psum
                        # closely (fine-grained tail on the last chunk)
                        for qg in range(QT128 // 4):
                            osb = pout.tile([P, 4, OCW], f32, tag="osb")
                            for qi in range(4):
                                qt = qg * 4 + qi
                                ops = ps.tile([P, OCW], f32, tag="ps",
                                              name=f"ops{c}_{qt}")
                                for kt in range(KT):
                                    nc.tensor.matmul(
                                        ops[:],
                                        p_r[kt][:, bass.ds(qt * P, P)],
                                        v_res[:, kt, ncol],
                                        start=(kt == 0),
                                        stop=(kt == KT - 1))
                                nc.vector.tensor_copy(osb[:, qi], ops[:])
                            if c == NOC - 1:
                                for qi in range(4):
                                    nc.gpsimd.dma_start(
                                        outv[:, qg * 4 + qi:
                                             qg * 4 + qi + 1, ncol],
                                        osb[:, qi:qi + 1])
                            else:
                                nc.gpsimd.dma_start(
                                    outv[:, qg * 4:(qg + 1) * 4, ncol],
                                    osb[:])

            if rep_cm is not None:
                rep_cm.__exit__(None, None, None)

    nc.compile()
    return nc


_CACHE = {}


def _get_nc(DX, N, DQ, DKH):
    key = (DX, N, DQ, DKH)
    if key not in _CACHE:
        _CACHE[key] = _build_core_kernel(DX, N, DQ, DKH)
    return _CACHE[key]


def _bf16(a):
    import ml_dtypes
    return np.ascontiguousarray(a.astype(ml_dtypes.bfloat16))


def _run(x, Wq, Wk, Wv, **spmd_kwargs):
    """Run the SPMD kernel; returns (out, BassKernelResults)."""
    from concourse.bass_utils import run_bass_kernel_spmd

    B, DX, N = x.shape
    DQ = Wq.shape[0]
    DK = Wk.shape[0]
    assert (B, DX, N, DQ, DK) == (B_FULL, DX_FULL, N_FULL, DQ_FULL, DK_FULL)
    DKH = DK // 2
    P = 128

    nc = _get_nc(DX, N, DQ, DKH)

    # Wq/Wk shipped mean-removed (entries - 0.5) in bf16. Q needs no
    # restore (cancels in softmax); K's restore is the host-exact
    # sh = 0.5*colsum(x), laid out [p, ntile].
    WqT = _bf16(np.ascontiguousarray(Wq.T, dtype=np.float32)
                - np.float32(0.5))
    WkT = _bf16(np.ascontiguousarray(Wk.T, dtype=np.float32)
                - np.float32(0.5))
    WvT = _bf16(np.ascontiguousarray(Wv.T, dtype=np.float32))

    in_maps = []
    for c in range(N_CORES):
        b, h = divmod(c, 2)
        hsl = slice(h * DKH, (h + 1) * DKH)
        s = (0.5 * x[b].astype(np.float32).sum(axis=0)).astype(np.float32)
        in_maps.append({
            "xb": _bf16(x[b]),
            "wqt": WqT,
            "wkt": np.ascontiguousarray(WkT[:, hsl]),
            "wvt": np.ascontiguousarray(WvT[:, hsl]),
            "sh": np.ascontiguousarray(s.reshape(N // P, P).T),
            "seed": np.zeros((1, 1), np.float32),
        })

    res = run_bass_kernel_spmd(nc, in_maps, core_ids=list(range(N_CORES)),
                               **spmd_kwargs)
    out = np.empty((B, DQ, N), np.float32)
    for b in range(B):
        out[b] = res.results[2 * b]["out"] + res.results[2 * b + 1]["out"]
    return out, res


def kernel(x, Wq, Wk, Wv):
    return _run(x, Wq, Wk, Wv)[0]
